# revision 98
# baseline (speedup 1.0000x reference)
"""Causal attention block kernel for TRN2, 8 NeuronCores.

Sharding: 8 cores = 4 batches x 2 head-groups (8 heads each).
Each core computes, for its (batch, head-group):
  qkv = x @ w_qkv + b_qkv ; causal softmax attention ; partial out-proj.
Host sums the two head-group partials per batch and adds b_out.

Per-core layout (q-partition flash attention):
  X^T [d,s] host pre-transposed; Q^T,K^T [64e, 2048s] per head (bf16);
  V augmented [s, (h, 64e + ones)] (bf16).  Per head, per k-chunk kc:
  S^T[k,q] = K^T.T @ Q^T in PSUM, E = exp(S/8) -> SBUF bf16 (wide ACT ops,
  compact triangular storage), causal mask of the diagonal 128x128 block
  into a separate tile.  PV runs q-tile-outer in q-partition layout:
  O[q,65] = sum_kc E_kc[:, qtile].T @ V_kc (ones column gives the softmax
  denominator in column 64), one short-lived PSUM bank per q-tile (an
  accumulation group must own a whole bank - start= clears it).
  Normalize with reciprocal + per-partition tensor_scalar (denominator
  lives on the q partition axis), PE-transpose head pairs back to [he, q],
  out-projection accumulates over head pairs into [q, d] PSUM.
  The whole emission is software-pipelined: scores run ~7 k-chunks ahead
  of PV, and the QKV projection is drip-fed into the attention stream on
  a deadline schedule so the PE always has filler while ACT runs the
  exp chain.
"""

import numpy as np
from contextlib import ExitStack

import concourse.bacc as bacc
import concourse.bass as bass
import concourse.mybir as mybir
import concourse.tile as tile
from concourse import bass_utils

F32 = mybir.dt.float32
BF16 = mybir.dt.bfloat16
AF = mybir.ActivationFunctionType

B, S, D, H, DH = 4, 2048, 1024, 16, 64
DEBUG = False
HPC = 8            # heads per core
NP = 4             # head pairs per core
NS = S // 128      # 16 s-tiles / k-chunks
NQ = S // 512      # 4 q-blocks
NDC = D // 128     # 8 d-chunks



def _emit(ctx: ExitStack, tc: tile.TileContext, io):
    nc = tc.nc
    x_d, wall_d, bqk_d, bvb_d, wo_d, tri_d, id_d, out_d = io[:8]

    const = ctx.enter_context(tc.tile_pool(name="const", bufs=1))

    # X^T [d, s] (host pre-transposed), interleaved with the fused
    # (wq|wk|wv) weight wall so projection chunks unblock early.  The
    # small constants slot in after the first two chunk pairs: they are
    # not needed until the first projection groups drain (~20us in).
    xt = [const.tile([128, S], BF16, tag=f"xt{dc}", name=f"xt{dc}") for dc in range(NDC)]
    wall = [const.tile([128, 1536], BF16, tag=f"wall{dc}", name=f"wall{dc}")
            for dc in range(NDC)]
    bqk = const.tile([128, 8], F32, tag="bqk", name="bqk")
    bvb = const.tile([128, HPC * DH], F32, tag="bvb", name="bvb")
    tri = const.tile([128, 128], BF16, tag="tri", name="tri")
    ident = const.tile([128, 128], BF16, tag="ident", name="ident")
    for dc in range(NDC):
        nc.sync.dma_start(xt[dc][:], x_d[dc * 128:(dc + 1) * 128, :])
        nc.sync.dma_start(wall[dc][:], wall_d[dc * 128:(dc + 1) * 128, :])
        if dc == 1:
            nc.sync.dma_start(bqk[:], bqk_d[:])
            nc.sync.dma_start(bvb[:], bvb_d[:])
            nc.sync.dma_start(tri[:], tri_d[:])
            nc.sync.dma_start(ident[:], id_d[:])
    wo = [const.tile([128, D], BF16, tag=f"wo{j}", name=f"wo{j}") for j in range(NP)]
    for j in range(NP):
        nc.sync.dma_start(wo[j][:], wo_d[j * 128:(j + 1) * 128, :])

    # persistent attention tensors
    qkt_pool = ctx.enter_context(tc.tile_pool(name="qkt", bufs=1))
    v3_pool = ctx.enter_context(tc.tile_pool(name="v3", bufs=1))
    oo_pool = ctx.enter_context(tc.tile_pool(name="oo", bufs=1))
    op_pool = ctx.enter_context(tc.tile_pool(name="opair", bufs=1))
    rc_pool = ctx.enter_context(tc.tile_pool(name="rc", bufs=8))
    # compact triangular E storage: chunk kc holds columns [kc*128, S),
    # resident for the whole head (PV is q-tile-outer).  Early chunks are
    # double-buffered so the next head's scores can run several chunks
    # ahead of this head's last PV q-tiles (pipeline skew).
    e_pool = ctx.enter_context(tc.tile_pool(name="epool", bufs=1))
    e_pool2 = ctx.enter_context(tc.tile_pool(name="epool2", bufs=2))
    em_pool = ctx.enter_context(tc.tile_pool(name="em", bufs=11))
    ob_pool = ctx.enter_context(tc.tile_pool(name="ob", bufs=3))

    qt = [qkt_pool.tile([128, S], BF16, tag=f"qt{j}", name=f"qt{j}") for j in range(NP)]
    kt = [qkt_pool.tile([128, S], BF16, tag=f"kt{j}", name=f"kt{j}") for j in range(NP)]
    v3 = [v3_pool.tile([128, HPC * 65], BF16, tag=f"v3_{st}", name=f"v3_{st}")
          for st in range(NS)]
    oo = [oo_pool.tile([128, S], BF16, tag=f"oo{j}", name=f"oo{j}") for j in range(NP)]
    opair = [op_pool.tile([128, 128], BF16, tag=f"opr{t}", name=f"opr{t}")
             for t in range(NS)]

    # single PSUM footprint for the whole kernel (8 banks):
    #   pst: 2x[128,1024] wide score slots (also recycled by the out-proj)
    #   psm: 2x one-bank slots - narrow late-kc scores, PE transposes,
    #        and drip-fed projection groups (deepens the exp pipeline)
    #   pop: 2 rotating PV accumulator banks (each accumulation group must
    #        own a full bank: start= clears the whole bank)
    pst = ctx.enter_context(tc.tile_pool(name="pst", bufs=2, space="PSUM"))
    psm = ctx.enter_context(tc.tile_pool(name="psm", bufs=2, space="PSUM"))
    pop = ctx.enter_context(tc.tile_pool(name="pop", bufs=2, space="PSUM"))

    # ---- projection group emitters ----
    def emit_v_group(st, pool=None, tag="st"):
        ps = (pool or pst).tile([128, HPC * DH], F32, tag=tag, name="psv")
        for dc in range(NDC):
            nc.tensor.matmul(
                ps[:], xt[dc][:, st * 128:(st + 1) * 128], wall[dc][:, 1024:1536],
                start=(dc == 0), stop=(dc == NDC - 1))
        v_view = v3[st][:].rearrange("p (h e) -> p h e", h=HPC)
        nc.vector.tensor_add(
            v_view[:, :, 0:DH],
            ps[:].rearrange("p (h e) -> p h e", h=HPC),
            bvb[:].rearrange("p (h e) -> p h e", h=HPC))
        nc.vector.memset(v_view[:, :, DH:65], 1.0)

    def emit_qk_group(mt, nb, pool=None, tag="st"):
        t, j = mt // NP, mt % NP
        dst = qt[j] if t == 0 else kt[j]
        ps = (pool or pst).tile([128, 512], F32, tag=tag, name="psqk")
        for dc in range(NDC):
            nc.tensor.matmul(
                ps[:], wall[dc][:, mt * 128:(mt + 1) * 128],
                xt[dc][:, nb * 512:(nb + 1) * 512],
                start=(dc == 0), stop=(dc == NDC - 1))
        nc.vector.tensor_scalar_add(
            dst[:, nb * 512:(nb + 1) * 512], ps[:], bqk[:, mt:mt + 1])

    # upfront: just enough for head 0 to start, spread across all idle
    # psum bank tags (attention hasn't claimed them yet)
    emit_qk_group(4, 0, tag="st")
    emit_qk_group(0, 0, tag="st")
    emit_qk_group(0, 1, pool=psm, tag="sm")
    emit_qk_group(0, 2, pool=psm, tag="sm")
    emit_qk_group(0, 3, pool=pop, tag="pv")
    emit_v_group(0, pool=pop, tag="pv")
    emit_v_group(1, tag="st")

    # everything else drip-feeds into the attention instruction stream,
    # paced so each projection lands just before its deadline (kt/qt pair p
    # before head 2p; v3[st] early in head 0) and fills PE idle in the
    # otherwise exp-paced middle heads.  Injected groups use the "sm" slots
    # so both score slots stay free to run the exp pipeline ahead.
    deferred = [("qk", 4, 1), ("v", 2, 0), ("qk", 4, 2), ("v", 3, 0),
                ("qk", 4, 3), ("v", 4, 0)]
    deferred += [("v", st, 0) for st in range(5, NS)]
    # per pair jj (heads 2jj/2jj+1 start at iter 32*jj): qt groups must all
    # land just before the pair's first head; kt columns stream in with kc.
    for jj in (1, 2, 3):
        deferred += [("qk", jj, nb) for nb in range(NQ)]
        deferred += [("qk", NP + jj, nb) for nb in range(NQ)]
    deferred.reverse()  # pop() from the end
    # injection slots: iter -> count (deadline-aware, as late as legal so PE
    # filler lands inside the exp-paced attention middle)
    _sched = {i: 2 for i in range(1, 9)}
    _sched[9] = 1
    for base in (32, 64, 96):              # pair jj = base//32 starts at base
        for i in (-14, -11, -8, -5):       # qt groups: all before the pair
            _sched[base + i] = 1
        for i in (-4, 0, 4, 8):            # kt group nb=k needed by base+4k
            _sched[base + i] = 1

    def inject(n):
        for _ in range(n):
            if not deferred:
                return
            kind, a, b = deferred.pop()
            if kind == "qk":
                emit_qk_group(a, b, pool=psm, tag="sm")
            else:
                emit_v_group(a, pool=psm, tag="sm")

    pair12 = {}  # carries the shared kc=12/13 psum + e tiles across iters

    def emit_scores(h, kc, e_of, em_of):
        """S^T chunk -> exp -> compact e (cols kc*128..S) + masked diag em.
        Chunks 12 and 13 share one 2-bank psum tile and one exp call (their
        pieces land in different banks, so both start= clears are safe)."""
        j, po = h // 2, (h % 2) * 64
        c0 = kc * 128
        kslice = kt[j][po:po + 64, kc * 128:(kc + 1) * 128]

        def mask(dst_col):
            em = em_pool.tile([128, 128], BF16, tag="em", name="em")
            nc.vector.tensor_mul(em[:], dst_col, tri[:])
            return em

        def second_seg():
            # [1024, 2048) segment for kc 4..7: own tile + exp as usual
            pool = e_pool2 if kc < 6 else e_pool
            e2 = pool.tile([128, 1024], BF16, tag=f"e{kc}b", name=f"e{kc}b")
            st_ps = pst.tile([128, 1024], F32, tag="st", name="st")
            for p0 in (1024, 1536):
                nc.tensor.matmul(st_ps[:, p0 - 1024:p0 - 512], kslice,
                                 qt[j][po:po + 64, p0:p0 + 512],
                                 start=True, stop=True)
            nc.scalar.activation(e2[:, 0:1024], st_ps[:, 0:1024],
                                 AF.Exp, scale=0.125)
            return e2

        # merged-exp pairs: the opener's piece clears its bank (start=True);
        # the closer lands in a fresh bank (start=True) or plain-writes the
        # already-cleared remainder of the same bank (start=False), and one
        # exp covers both chunks.
        # kc: (partner_off, psum pool/tag/width, e pool/tag/width, fresh_bank)
        _PAIR = {4: (512, "A"), 6: (256, "B"), 12: (512, "A"), 14: (256, "B")}
        if kc in _PAIR:   # opener
            off, kind = _PAIR[kc]
            w = off  # opener piece width == partner offset
            if kind == "A":
                psf = pst.tile([128, 1024], F32, tag="st", name=f"stp{kc}")
                ef = (e_pool2 if kc == 4 else e_pool).tile(
                    [128, 896], BF16, tag=f"ep{kc}", name=f"ep{kc}")
            else:
                psf = psm.tile([128, 512], F32, tag="sm", name=f"smp{kc}")
                ef = (e_pool2 if kc == 6 else e_pool).tile(
                    [128, 384], BF16, tag=f"ep{kc}", name=f"ep{kc}")
            lim = 1024 if kc < 8 else 2048
            nc.tensor.matmul(psf[:, 0:w], kslice, qt[j][po:po + 64, c0:lim],
                             start=True, stop=True, skip_group_check=True)
            pair12[kc] = (psf, ef)
            e_of[kc] = [(c0, ef[:, 0:w])]
            em_of[kc] = None  # produced at the closer (patched into snapshots)
            if kc < 8:
                e_of[kc].append((1024, second_seg()[:]))
            return
        if kc - 1 in _PAIR:  # closer
            off, kind = _PAIR[kc - 1]
            psf, ef = pair12[kc - 1]
            lim = 1024 if kc < 8 else 2048
            w = lim - c0
            nc.tensor.matmul(psf[:, off:off + w], kslice,
                             qt[j][po:po + 64, c0:lim],
                             start=(kind == "A"), stop=True,
                             skip_group_check=True)
            nc.scalar.activation(ef[:, 0:off + w], psf[:, 0:off + w],
                                 AF.Exp, scale=0.125)
            em_of[kc - 1] = mask(ef[:, 0:128])
            em_of[kc] = mask(ef[:, off:off + 128])
            e_of[kc] = [(c0, ef[:, off:off + w])]
            if kc < 8:
                e_of[kc].append((1024, second_seg()[:]))
            return

        pool = e_pool2 if kc < 6 else e_pool
        e_t = pool.tile([128, S - c0], BF16, tag=f"e{kc}", name=f"e{kc}")
        seg0 = c0
        while seg0 < S:
            segw = min(1024 - seg0 % 1024, S - seg0)
            st_ps = pst.tile([128, 1024], F32, tag="st", name="st")
            sb = seg0 % 1024
            p0 = seg0
            while p0 < seg0 + segw:
                pw = min(512 - p0 % 512, seg0 + segw - p0)
                nc.tensor.matmul(
                    st_ps[:, p0 - seg0 + sb:p0 - seg0 + sb + pw],
                    kslice,
                    qt[j][po:po + 64, p0:p0 + pw],
                    start=True, stop=True)
                p0 += pw
            nc.scalar.activation(
                e_t[:, seg0 - c0:seg0 - c0 + segw], st_ps[:, sb:sb + segw],
                AF.Exp, scale=0.125)
            seg0 += segw
        if DEBUG and h == 0 and kc == 0:
            nc.sync.dma_start(io[-1][4][0:128, :], e_t[:])
        e_of[kc] = [(c0, e_t[:])]
        em_of[kc] = mask(e_t[:, 0:128])

    def emit_outproj(mts):
        for mt in mts:
            ps = pst.tile([128, D], F32, tag="st", name="psz")
            if mt >= 14:
                # kernel tail: store per 512-half so the copy+DMA of half 0
                # pipelines with half 1's matmuls
                for nb in range(2):
                    for jj in range(NP):
                        nc.tensor.matmul(
                            ps[:, nb * 512:(nb + 1) * 512],
                            oo[jj][:, mt * 128:(mt + 1) * 128],
                            wo[jj][:, nb * 512:(nb + 1) * 512],
                            start=(jj == 0), stop=(jj == NP - 1))
                    obh = ob_pool.tile([128, 512], BF16, tag="obh", name="obh")
                    nc.scalar.copy(obh[:], ps[:, nb * 512:(nb + 1) * 512])
                    nc.sync.dma_start(
                        out_d[mt * 128:(mt + 1) * 128,
                              nb * 512:(nb + 1) * 512], obh[:])
                continue
            for nb in range(2):
                for jj in range(NP):
                    nc.tensor.matmul(
                        ps[:, nb * 512:(nb + 1) * 512],
                        oo[jj][:, mt * 128:(mt + 1) * 128],
                        wo[jj][:, nb * 512:(nb + 1) * 512],
                        start=(jj == 0), stop=(jj == NP - 1))
            ob = ob_pool.tile([128, D], BF16, tag="ob", name="ob")
            nc.scalar.copy(ob[:], ps[:])
            nc.sync.dma_start(out_d[mt * 128:(mt + 1) * 128, :], ob[:])

    def emit_pv(h, t, e_of, em_of):
        """PV for q-tile t: one short-lived accumulation group per bank,
        then normalize (and transpose once the head pair is complete)."""
        j, po = h // 2, (h % 2) * 64
        ps = pop.tile([128, 65], F32, tag="pv", name="pv")
        q0 = t * 128
        for kc in range(t + 1):
            if kc == t:
                lhsT = em_of[kc][:]
            else:
                b, ap = next(s for s in reversed(e_of[kc]) if s[0] <= q0)
                lhsT = ap[:, q0 - b:q0 - b + 128]
            nc.tensor.matmul(
                ps[:], lhsT, v3[kc][:, h * 65:(h + 1) * 65],
                start=(kc == 0), stop=(kc == t))
        r = rc_pool.tile([128, 1], F32, tag="rc", name="rc")
        nc.vector.reciprocal(r[:], ps[:, 64:65])
        nc.vector.tensor_scalar_mul(
            opair[t][:, po:po + 64], ps[:, 0:DH], r[:])
        if po:  # pair complete: transpose back to [he, q] for out-proj
            tps = psm.tile([128, 128], BF16, tag="sm", name="tps")
            nc.tensor.transpose(tps[:], opair[t][:], ident[:])
            # oo copies on DVE (cheap 2x bf16 path) so the drain's two copy
            # links (oo and ob) run on different engines
            nc.vector.tensor_copy(oo[j][:, t * 128:(t + 1) * 128], tps[:])
            if h == HPC - 1:
                emit_outproj([t])  # all pairs done: out-proj for this q-tile

    # ---- attention, software-pipelined four k-chunks ahead ----
    from collections import deque
    pending = deque()
    kc_iter = 0
    e_of, em_of = [None] * NS, [None] * NS
    for h in range(HPC):
        for kc in range(NS):
            inject(_sched.get(kc_iter, 0))
            kc_iter += 1
            emit_scores(h, kc, e_of, em_of)
            if kc in (5, 7, 13, 15):
                i = kc - 1  # patch the paired chunk's deferred mask into
                for p in pending:  # snapshots taken before it existed
                    if p[0] == h and p[3][i] is None:
                        p[3][i] = em_of[i]
            pending.append((h, kc, list(e_of), list(em_of)))
            lim = 7
            while len(pending) > lim:
                emit_pv(*pending.popleft())
    while pending:
        emit_pv(*pending.popleft())

    if DEBUG:
        dbg_qt, dbg_kt, dbg_v3, dbg_oo, dbg_e, dbg_op = io[-1]
        for t in range(NS):
            nc.sync.dma_start(dbg_op[t * 128:(t + 1) * 128, :], opair[t][:])
        for jj in range(NP):
            nc.sync.dma_start(dbg_qt[jj * 128:(jj + 1) * 128, :], qt[jj][:])
            nc.sync.dma_start(dbg_kt[jj * 128:(jj + 1) * 128, :], kt[jj][:])
            nc.sync.dma_start(dbg_oo[jj * 128:(jj + 1) * 128, :], oo[jj][:])
        for st in range(NS):
            nc.sync.dma_start(dbg_v3[st * 128:(st + 1) * 128, :], v3[st][:])


def _build():
    nc = bacc.Bacc("TRN2", target_bir_lowering=False, debug=False)
    x_d = nc.dram_tensor("x_s", [D, S], BF16, kind="ExternalInput").ap()
    wall_d = nc.dram_tensor("wall", [D, 1536], BF16, kind="ExternalInput").ap()
    bqk_d = nc.dram_tensor("bqk", [128, 8], F32, kind="ExternalInput").ap()
    bvb_d = nc.dram_tensor("bvb", [128, HPC * DH], F32, kind="ExternalInput").ap()
    wo_d = nc.dram_tensor("wo", [HPC * DH, D], BF16, kind="ExternalInput").ap()
    tri_d = nc.dram_tensor("tri", [128, 128], BF16, kind="ExternalInput").ap()
    id_d = nc.dram_tensor("ident", [128, 128], BF16, kind="ExternalInput").ap()
    out_d = nc.dram_tensor("out_s", [S, D], BF16, kind="ExternalOutput").ap()
    io = [x_d, wall_d, bqk_d, bvb_d, wo_d, tri_d, id_d, out_d]
    if DEBUG:
        dbg = (nc.dram_tensor("dbg_qt", [512, S], BF16, kind="ExternalOutput").ap(),
               nc.dram_tensor("dbg_kt", [512, S], BF16, kind="ExternalOutput").ap(),
               nc.dram_tensor("dbg_v3", [NS * 128, HPC * 65], BF16, kind="ExternalOutput").ap(),
               nc.dram_tensor("dbg_oo", [512, S], BF16, kind="ExternalOutput").ap(),
               nc.dram_tensor("dbg_e", [256, S], BF16, kind="ExternalOutput").ap(),
               nc.dram_tensor("dbg_op", [NS * 128, 128], BF16, kind="ExternalOutput").ap())
        io.append(dbg)
    with tile.TileContext(nc) as tc:
        with ExitStack() as ctx:
            _emit(ctx, tc, io)
    nc.compile()
    return nc


_NC = None


def _get_nc():
    global _NC
    if _NC is None:
        _NC = _build()
    return _NC


def _host_inputs(x, w_qkv, b_qkv, w_out):
    """Per-head-group shared weight arrays + per-core x."""
    import ml_dtypes
    maps = []
    hg_arrs = []
    for hg in range(2):
        hs = slice(hg * HPC, (hg + 1) * HPC)
        wq = np.asarray(w_qkv[:, 0, hs, :]).reshape(D, HPC * DH)
        wk = np.asarray(w_qkv[:, 1, hs, :]).reshape(D, HPC * DH)
        wv = np.asarray(w_qkv[:, 2, hs, :]).reshape(D, HPC * DH)
        wall = np.concatenate([wq, wk, wv], axis=1).astype(ml_dtypes.bfloat16)
        bq = np.asarray(b_qkv[0, hs, :]).reshape(HPC * DH)
        bk = np.asarray(b_qkv[1, hs, :]).reshape(HPC * DH)
        bqk = np.zeros((128, 8), np.float32)
        for mt in range(8):
            t, j = mt // NP, mt % NP
            src = bq if t == 0 else bk
            bqk[:, mt] = src[j * 128:(j + 1) * 128]
        bvb = np.broadcast_to(
            np.asarray(b_qkv[2, hs, :]).reshape(1, HPC * DH), (128, HPC * DH)
        ).astype(np.float32)
        wo = np.asarray(w_out[hs]).reshape(HPC * DH, D).astype(ml_dtypes.bfloat16)
        trim = (np.arange(128)[None, :] >= np.arange(128)[:, None]).astype(
            ml_dtypes.bfloat16)
        idm = np.eye(128, dtype=ml_dtypes.bfloat16)
        hg_arrs.append(dict(wall=wall, bqk=bqk, bvb=bvb, wo=wo,
                            tri=trim, ident=idm))
    for c in range(8):
        b, hg = c % B, c // B
        m = dict(hg_arrs[hg])
        m["x_s"] = np.ascontiguousarray(
            np.asarray(x[b]).astype(ml_dtypes.bfloat16).T)
        maps.append(m)
    return maps


def _run(inputs, trace=False, tmpdir=None):
    nc = _get_nc()
    in_maps = _host_inputs(inputs["x"], inputs["w_qkv"], inputs["b_qkv"],
                           inputs["w_out"])
    res = bass_utils.run_bass_kernel_spmd(
        nc, in_maps, core_ids=list(range(8)), trace=trace, tmpdir=tmpdir)
    b_out = np.asarray(inputs["b_out"], dtype=np.float32)
    out = np.empty((B, S, D), np.float32)
    for b in range(B):
        out[b] = (res.results[b]["out_s"].astype(np.float32)
                  + res.results[b + B]["out_s"].astype(np.float32)
                  + b_out[None, :])
    return out, res


def kernel(**inputs) -> np.ndarray:
    out, _ = _run(inputs, trace=False)
    return out
